# revision 7
# baseline (speedup 1.0000x reference)
import numpy as np

KSIZE = 21
DEPTH, WIDTH = 9, 100
KK = KSIZE * KSIZE  # 441

_PROG = None
LAST_DEVICE_SECONDS = None


def _build_program():
    import concourse.bass as bass
    import concourse.tile as tile
    import concourse.mybir as mybir
    from concourse import bacc

    F32 = mybir.dt.float32
    F32R = mybir.dt.float32r
    AF = mybir.ActivationFunctionType

    def mk_ap(t_ap, offset_el, dims):
        return bass.AP(tensor=t_ap.tensor, offset=offset_el,
                       ap=[[s, n] for (s, n) in dims])

    nc = bacc.Bacc(None, target_bir_lowering=False)
    x = nc.dram_tensor("x", [34, 82, 128], F32, kind="ExternalInput")
    wds = []
    for l in range(DEPTH):
        cin = 34 if l == 0 else WIDTH
        cout = KK if l == DEPTH - 1 else WIDTH
        wds.append(nc.dram_tensor(f"w{l}", [cin, 25, cout], F32,
                                  kind="ExternalInput"))
    y = nc.dram_tensor("y", [KK, 46, 92], F32, kind="ExternalOutput")

    groups = [(0, 128), (128, 128), (256, 128), (384, 57)]

    with tile.TileContext(nc) as tc:
        with tc.tile_pool(name="p", bufs=1) as pool, \
             tc.tile_pool(name="ps", bufs=4, space="PSUM") as psum:
            cur = pool.tile([34, 82, 128], F32R, tag="a0", bufs=1)
            nc.gpsimd.dma_start(cur[:], x[:])
            r_in, w_in = 82, 128
            for l in range(DEPTH - 1):
                cin = 34 if l == 0 else WIDTH
                wt = pool.tile([cin, 25, WIDTH], F32R, tag="wb", bufs=3,
                               name=f"wt{l}")
                nc.gpsimd.dma_start(wt[:], wds[l][:])
                r_out, w_out = r_in - 4, w_in - 4
                nxt = pool.tile([WIDTH, r_out, w_out], F32R,
                                tag=f"a{(l + 1) % 2}", bufs=1, name=f"act{l}")
                row = 0
                while row < r_out:
                    nr = min(4, r_out - row)
                    pt = psum.tile([WIDTH, nr, w_out], F32, name="pt",
                                   tag="pt", bufs=2)
                    for ky in range(5):
                        for kx in range(5):
                            rhs = mk_ap(cur[:], (row + ky) * w_in + kx,
                                        [(r_in * w_in, cin), (w_in, nr),
                                         (1, w_out)])
                            nc.tensor.matmul(pt[:], wt[:, ky * 5 + kx, :], rhs,
                                             start=(ky == 0 and kx == 0),
                                             stop=(ky == 4 and kx == 4))
                    nc.scalar.activation(nxt[:, row:row + nr, :], pt[:], AF.Relu)
                    row += nr
                cur = nxt
                r_in, w_in = r_out, w_out
            # layer 9: linear, 441 outputs in 4 channel groups
            r_out, w_out = r_in - 4, w_in - 4  # 46, 92
            for gi, (g0, gn) in enumerate(groups):
                wt = pool.tile([WIDTH, 25, gn], F32R, tag="wb", bufs=3,
                               padded_shape=[WIDTH, 25, 128], name=f"wt8_{gi}")
                nc.gpsimd.dma_start(wt[:], wds[8][:, :, g0:g0 + gn])
                row = 0
                while row < r_out:
                    nr = min(5, r_out - row)
                    pt = psum.tile([gn, nr, w_out], F32, name="pt",
                                   tag="pt", bufs=2)
                    for ky in range(5):
                        for kx in range(5):
                            rhs = mk_ap(cur[:], (row + ky) * w_in + kx,
                                        [(r_in * w_in, WIDTH), (w_in, nr),
                                         (1, w_out)])
                            nc.tensor.matmul(pt[:], wt[:, ky * 5 + kx, :], rhs,
                                             start=(ky == 0 and kx == 0),
                                             stop=(ky == 4 and kx == 4))
                    st = pool.tile([gn, 5, 92], F32, tag="st", bufs=2,
                                   padded_shape=[128, 5, 92],
                                   name=f"st{gi}_{row}")
                    nc.vector.tensor_copy(st[:, :nr, :], pt[:])
                    nc.sync.dma_start(y[g0:g0 + gn, row:row + nr, :],
                                      st[:, :nr, :])
                    row += nr
    nc.compile()
    return nc


def _get_prog():
    global _PROG
    if _PROG is None:
        _PROG = _build_program()
    return _PROG


def _tw(w):
    # [cout, cin, 5, 5] -> [cin, 25, cout]
    w = np.asarray(w, np.float32)
    return np.ascontiguousarray(w.transpose(1, 2, 3, 0).reshape(
        w.shape[1], 25, w.shape[0]))


def _chain_inputs(params):
    # returns per-chain weight dicts (layer-1 cin padded to 34)
    outs = []
    for name in ("Gd", "Gs", "Pd", "Ps"):
        layers = params[name]
        d = {}
        for l in range(DEPTH):
            wt = _tw(layers[l][0])
            if l == 0 and wt.shape[0] < 34:
                wt = np.concatenate(
                    [wt, np.zeros((34 - wt.shape[0], 25, wt.shape[2]),
                                  np.float32)], axis=0)
            d[f"w{l}"] = wt
        outs.append(d)
    return outs


def _run_chains(kdi, ksi, pathd, pathsp, params, trace=False):
    from concourse.bass_utils import run_bass_kernel_spmd
    nc = _get_prog()
    p_diff_in = np.concatenate([kdi[0, :10], pathd[0]], 0)
    p_spec_in = np.concatenate([ksi[0, :10], pathsp[0]], 0)
    pad = np.zeros((17, 128, 128), np.float32)
    xs = [kdi[0], ksi[0],
          np.concatenate([p_diff_in, pad], 0),
          np.concatenate([p_spec_in, pad], 0)]
    wmaps = _chain_inputs(params)
    in_maps = []
    for c in range(4):
        for h in range(2):
            rows = slice(0, 82) if h == 0 else slice(46, 128)
            m = dict(wmaps[c])
            m["x"] = np.ascontiguousarray(xs[c][:, rows, :], np.float32)
            in_maps.append(m)
    import time as _time
    t0 = _time.time()
    res = run_bass_kernel_spmd(nc, in_maps, core_ids=list(range(8)),
                               trace=trace)
    global LAST_DEVICE_SECONDS
    LAST_DEVICE_SECONDS = _time.time() - t0
    maps = []
    for c in range(4):
        full = np.concatenate(
            [res.results[2 * c]["y"], res.results[2 * c + 1]["y"]], axis=1)
        maps.append(full[None])  # [1,441,92,92]
    return maps, res


def _host_tail(maps, kdi, ksi, pathd, pathsp, bufd, bufs_, albedo, params):
    import jax
    import jax.numpy as jnp

    def _conv(x, w, b, stride=1, padding='VALID'):
        yy = jax.lax.conv_general_dilated(
            x, w, (stride, stride), padding,
            dimension_numbers=('NCHW', 'OIHW', 'NCHW'))
        return yy + b[None, :, None, None]

    def _conv_chain(x, layers, out_act, pad):
        padding = 'SAME' if pad else 'VALID'
        n = len(layers)
        for i, (w, b) in enumerate(layers):
            x = _conv(x, w, b, padding=padding)
            if i < n - 1:
                x = jax.nn.relu(x)
        if out_act == 'sigmoid':
            x = jax.nn.sigmoid(x)
        return x

    def _crop_like(src, ref):
        dh = (src.shape[2] - ref.shape[2]) // 2
        dw = (src.shape[3] - ref.shape[3]) // 2
        return src[:, :, dh:dh + ref.shape[2], dw:dw + ref.shape[3]]

    def _weighted_filter(buf, w):
        b, c, h, ww = buf.shape
        r = KSIZE // 2
        padb = jnp.pad(buf, ((0, 0), (0, 0), (r, r), (r, r)))
        patches = jax.lax.conv_general_dilated_patches(
            padb, (KSIZE, KSIZE), (1, 1), 'VALID',
            dimension_numbers=('NCHW', 'OIHW', 'NCHW'))
        patches = patches.reshape(b, c, KSIZE * KSIZE, h, ww)
        return jnp.einsum('bckhw,bkhw->bchw', patches, w)

    def _unet(x, p):
        e1 = jax.nn.relu(_conv(x, p['e1w'], p['e1b'], padding='SAME'))
        e2 = jax.nn.relu(_conv(e1, p['e2w'], p['e2b'], stride=2,
                               padding='SAME'))
        u = jnp.repeat(jnp.repeat(e2, 2, axis=2), 2, axis=3)
        u = jax.nn.relu(_conv(u, p['uw'], p['ub'], padding='SAME'))
        d = jax.nn.relu(_conv(jnp.concatenate([u, e1], 1), p['dw'], p['db'],
                              padding='SAME'))
        return _conv(d, p['ow'], p['ob'], padding='SAME')

    with jax.default_device(jax.devices("cpu")[0]):
        g_k_d, g_k_s, p_k_d, p_k_s = [jnp.asarray(m) for m in maps]
        feat_d = jnp.concatenate([kdi[:, 10:], pathd], 1)
        feat_s = jnp.concatenate([ksi[:, 10:], pathsp], 1)
        b_d = _crop_like(jnp.asarray(bufd), g_k_d)
        b_s = _crop_like(jnp.asarray(bufs_), g_k_s)
        g_r_d = _weighted_filter(b_d, jax.nn.softmax(g_k_d, axis=1))
        g_r_s = _weighted_filter(b_s, jax.nn.softmax(g_k_s, axis=1))
        p_r_d = _weighted_filter(b_d, jax.nn.softmax(p_k_d, axis=1))
        p_r_s = _weighted_filter(b_s, jax.nn.softmax(p_k_s, axis=1))
        fd = _crop_like(feat_d, g_r_d)
        fs = _crop_like(feat_s, g_r_s)
        prm = params
        dis_G_d = jax.nn.sigmoid(_unet(jnp.concatenate([g_r_d, fd], 1),
                                       prm['Dd']))
        dis_P_d = jax.nn.sigmoid(_unet(jnp.concatenate([p_r_d, fd], 1),
                                       prm['Dd']))
        dis_G_s = jax.nn.sigmoid(_unet(jnp.concatenate([g_r_s, fs], 1),
                                       prm['Ds']))
        dis_P_s = jax.nn.sigmoid(_unet(jnp.concatenate([p_r_s, fs], 1),
                                       prm['Ds']))
        w_d = _conv_chain(jnp.concatenate([dis_G_d[:, 1:], dis_P_d[:, 1:]], 1),
                          prm['Rd'], 'sigmoid', True)
        w_s = _conv_chain(jnp.concatenate([dis_G_s[:, 1:], dis_P_s[:, 1:]], 1),
                          prm['Rs'], 'sigmoid', True)
        n_k_d = g_k_d * w_d + p_k_d * (1.0 - w_d)
        n_k_s = g_k_s * w_s + p_k_s * (1.0 - w_s)
        r_d = _weighted_filter(b_d, jax.nn.softmax(n_k_d, axis=1))
        r_s = _weighted_filter(b_s, jax.nn.softmax(n_k_s, axis=1))
        alb = _crop_like(jnp.asarray(albedo), r_d)
        out = alb * r_d + (jnp.exp(r_s) - 1.0)
        return np.asarray(out, np.float32)


def kernel(kpcn_diffuse_in, kpcn_specular_in, paths_diffuse, paths_specular,
           kpcn_diffuse_buffer, kpcn_specular_buffer, kpcn_albedo,
           target_diffuse, target_specular, params, _trace=False, _res=None):
    kdi = np.asarray(kpcn_diffuse_in, np.float32)
    ksi = np.asarray(kpcn_specular_in, np.float32)
    pathd = np.asarray(paths_diffuse, np.float32)
    pathsp = np.asarray(paths_specular, np.float32)
    maps, res = _run_chains(kdi, ksi, pathd, pathsp, params, trace=_trace)
    if _res is not None:
        _res.append(res)
    return _host_tail(maps, kdi, ksi, pathd, pathsp,
                      np.asarray(kpcn_diffuse_buffer, np.float32),
                      np.asarray(kpcn_specular_buffer, np.float32),
                      np.asarray(kpcn_albedo, np.float32), params)
